# revision 26
# baseline (speedup 1.0000x reference)
"""Trainium2 Bass kernel for nn_Chemical_feature_interaction.

Math (per sample b):
    u = x1 @ var_1.T                  # [B, 32]
    v = x2 @ var_2                    # [B, 32]
    x3[b, c*32+r] = v[b,c] * u[b,r]   # [B, 1024]
    out = MLP(x3)  (1024->512->128->32->1, ReLU between, bias everywhere)

v3 dataflow (per core, feature-major, fp16 operands, fp32 PSUM):
  - batch sharded 8 ways (4096 rows/core), chunks of NB=512 batch columns.
  - U4 = tile(u,4) [128, NB] from one matmul chain against the tiled
    var1 stationary (M=128 costs the same as M=32).
  - iv [128, NB] from the v-matmul against a host-PERMUTED var2
    stationary: iv[32g+k] = v[4k+g].  Each V_k [128, NB] (= v[4k+p//32])
    is then ONE DVE stream_shuffle with mask [k]*32 — no PE selector
    matmuls (K=32 matmuls are also hw-penalized ~1.7x vs K=128).
  - x3[:,k,:] = V_k * U4 alternating DVE / Pool engines.
  - W1 m-outer (each m-block's ReLU overlaps the next block's matmuls),
    ReLU+bias fused in the PSUM->SBUF copy on ACT.
  - all input DMA pre-emitted in priority order (DMA is ~one serial
    358 GB/s resource): v1r, x1t0, v2p, x2t0, x1t1, w1-m0, x2t1, rest;
    x1 rides the SP ring, x2 the SWDGE queue, weights the ACT ring.
    w1 is m-major in dram so m-blocks land in consumption order.
  - last two chunks' tails interleaved so the ACT<->PE ping-pong of the
    final W2/W3/W4 stage hides under the other chunk's matmuls.
  - out stored per chunk as a contiguous [1, NB] f32 row (out dram is
    (1, BL); host reshapes).

PE work: 54 matmuls/chunk x ~(512+128+~40)cyc @2.4GHz ~= 15.3us/chunk,
8 chunks ~= 123us + edges.  (fp16 selector baseline: 164.6us measured;
v2: 136.5us measured.)
"""

import numpy as np

import concourse.bacc as bacc
import concourse.mybir as mybir
import concourse.tile as tile
from concourse import bass_utils

B = 32768
D = 1024
R = 32
NCORES = 8
BL = B // NCORES  # rows per core
NB = 512  # batch columns per chunk
NCHUNK = BL // NB
KC = D // 128  # k-chunks of the 1024 contraction dims

F16 = mybir.dt.float16
F32 = mybir.dt.float32
AF = mybir.ActivationFunctionType

CFG = {
    "tail_split": True,   # 3-stage tail pipeline vs single tail
    "store_eng": "sync",  # sync | gpsimd
    "startup": "swdge",   # swdge | rings
    "x2t1_first": True,
}


def _build(b4_val: float, repeat: int = 1):
    """Build the per-core Bass program. repeat>1 wraps the kernel in a
    device-side loop (benchmarking only)."""
    nc = bacc.Bacc("TRN2", target_bir_lowering=False, debug=False)

    # x1/x2 host-pre-transposed and chunk-blocked: [NCHUNK, 128, KC, NB],
    # [i, p, k, b] = x[i*NB + b, 128k + p] — one contiguous 1 MB chunk tile.
    x1_d = nc.dram_tensor("x1", (NCHUNK, 128, KC, NB), F16,
                          kind="ExternalInput").ap()
    x2_d = nc.dram_tensor("x2", (NCHUNK, 128, KC, NB), F16,
                          kind="ExternalInput").ap()
    v1r_d = nc.dram_tensor("v1r", (128, KC, 128), F16, kind="ExternalInput").ap()
    v2p_d = nc.dram_tensor("v2p", (128, KC, 128), F16, kind="ExternalInput").ap()
    w1_d = nc.dram_tensor("w1l", (4, 128, KC, 128), F16,
                          kind="ExternalInput").ap()
    w2_d = nc.dram_tensor("w2l", (128, 4, 128), F16, kind="ExternalInput").ap()
    w3_d = nc.dram_tensor("w3l", (128, 128), F16, kind="ExternalInput").ap()
    w4_d = nc.dram_tensor("w4l", (128, 1), F16, kind="ExternalInput").ap()
    b1_d = nc.dram_tensor("b1l", (128, 4), F32, kind="ExternalInput").ap()
    b2_d = nc.dram_tensor("b2l", (128, 1), F32, kind="ExternalInput").ap()
    b3_d = nc.dram_tensor("b3l", (128, 1), F32, kind="ExternalInput").ap()
    out_d = nc.dram_tensor("out", (1, BL), F32, kind="ExternalOutput").ap()

    with tile.TileContext(nc) as tc:
        with (
            tc.tile_pool(name="consts", bufs=1) as consts,
            tc.tile_pool(name="io", bufs=4) as io,
            tc.tile_pool(name="work", bufs=2) as work,
            tc.tile_pool(name="psum", bufs=2, space="PSUM") as psum,
        ):
            v1r_sb = consts.tile([128, KC, 128], F16)
            v2p_sb = consts.tile([128, KC, 128], F16)
            w1_sb = consts.tile([128, 4, KC, 128], F16)
            w2_sb = consts.tile([128, 4, 128], F16)
            w3_sb = consts.tile([128, 128], F16)
            w4_sb = consts.tile([128, 1], F16)
            b1_sb = consts.tile([128, 4], F32)
            b2_sb = consts.tile([128, 1], F32)
            b3_sb = consts.tile([128, 1], F32)

            consts_sb = dict(v1r=v1r_sb, v2p=v2p_sb, w1=w1_sb, w2=w2_sb,
                             w3=w3_sb, w4=w4_sb, b1=b1_sb, b2=b2_sb,
                             b3=b3_sb)
            consts_d = dict(v1r=v1r_d, v2p=v2p_d, w1=w1_d, w2=w2_d,
                            w3=w3_d, w4=w4_d, b1=b1_d, b2=b2_d, b3=b3_d)

            def body():
                _chunks(nc, x1_d, x2_d, out_d, consts_sb, consts_d, b4_val,
                        io, work, psum)

            if repeat == 1:
                body()
            else:
                ET = mybir.EngineType
                with tc.For_i(0, repeat, 1,
                              hint_engines=(ET.PE, ET.Activation, ET.SP,
                                            ET.DVE, ET.Pool)):
                    body()

    nc.compile()
    return nc


def _chunks(nc, x1_d, x2_d, out_d, csb, cd, b4_val, io, work, psum):
    """Emit input DMA (priority-ordered), then the 8 software-pipelined
    batch chunks (head(i+1) emitted before tail(i))."""

    # ---- input DMA, global priority order (shared serial DMA resource).
    # SP ring: x1 chunks; ACT ring: weights; SWDGE: x2 chunks + stores.
    x1ts, x2ts = [], []
    for i in range(NCHUNK):
        x1ts.append(io.tile([128, KC, NB], F16, tag="x1t", name=f"x1t{i}"))
        x2ts.append(io.tile([128, KC, NB], F16, tag="x2t", name=f"x2t{i}"))

    # startup-critical bytes via SWDGE (bypasses the serial HWDGE issue
    # track); chunk 1 / w1 / consts on the HWDGE rings behind them
    if CFG["startup"] == "swdge":
        nc.gpsimd.dma_start(csb["v1r"][:], cd["v1r"][:])
        nc.gpsimd.dma_start(x1ts[0][:, 0:4, :], x1_d[0, :, 0:4, :])
        nc.gpsimd.dma_start(csb["v2p"][:], cd["v2p"][:])
        nc.gpsimd.dma_start(x2ts[0][:, 0:4, :], x2_d[0, :, 0:4, :])
        nc.gpsimd.dma_start(x1ts[0][:, 4:8, :], x1_d[0, :, 4:8, :])
        nc.gpsimd.dma_start(x2ts[0][:, 4:8, :], x2_d[0, :, 4:8, :])
    else:
        nc.sync.dma_start(csb["v1r"][:], cd["v1r"][:])
        nc.scalar.dma_start(csb["v2p"][:], cd["v2p"][:])
        nc.sync.dma_start(x1ts[0][:, 0:4, :], x1_d[0, :, 0:4, :])
        nc.scalar.dma_start(x2ts[0][:, 0:4, :], x2_d[0, :, 0:4, :])
        nc.sync.dma_start(x1ts[0][:, 4:8, :], x1_d[0, :, 4:8, :])
        nc.scalar.dma_start(x2ts[0][:, 4:8, :], x2_d[0, :, 4:8, :])
    if CFG["x2t1_first"]:
        nc.sync.dma_start(x1ts[1][:], x1_d[1])
        nc.scalar.dma_start(x2ts[1][:], x2_d[1])
        nc.scalar.dma_start(csb["w1"][:, 0], cd["w1"][0])
    else:
        nc.sync.dma_start(x1ts[1][:], x1_d[1])
        nc.scalar.dma_start(csb["w1"][:, 0], cd["w1"][0])
        nc.scalar.dma_start(x2ts[1][:], x2_d[1])
    for m in range(1, 4):
        nc.scalar.dma_start(csb["w1"][:, m], cd["w1"][m])
    for name in ("b1", "w2", "w3", "w4", "b2", "b3"):
        nc.scalar.dma_start(csb[name][:], cd[name][:])
    for i in range(2, NCHUNK):
        nc.sync.dma_start(x1ts[i][:], x1_d[i])
        nc.scalar.dma_start(x2ts[i][:], x2_d[i])

    # ---- compute
    w1_sb, w2_sb, w3_sb, w4_sb = csb["w1"], csb["w2"], csb["w3"], csb["w4"]
    b1_sb, b2_sb, b3_sb = csb["b1"], csb["b2"], csb["b3"]

    def head(i):
        x1t, x2t = x1ts[i], x2ts[i]
        u4_ps = psum.tile([128, NB], F32, tag="uv", bufs=2, name="u4ps")
        iv_ps = psum.tile([128, NB], F32, tag="uv", bufs=2, name="ivps")
        # k-interleaved accumulation: consumes x slices in DMA arrival
        # order, so chunk-0 compute tracks the incoming stream
        for k in range(KC):
            nc.tensor.matmul(u4_ps, csb["v1r"][:, k, :], x1t[:, k, :],
                             start=(k == 0), stop=(k == KC - 1))
            nc.tensor.matmul(iv_ps, csb["v2p"][:, k, :], x2t[:, k, :],
                             start=(k == 0), stop=(k == KC - 1))

        # keep ACT pure-Relu (function switches cost ~1.3us table reloads)
        u4_sb = work.tile([128, NB], F16, tag="u4")
        iv_sb = work.tile([128, NB], F16, tag="iv")
        nc.vector.tensor_copy(iv_sb, iv_ps)
        nc.vector.tensor_copy(u4_sb, u4_ps)

        x3 = work.tile([128, KC, NB], F16, tag="x3")
        for k in range(KC):
            # V_k[32g+j] = iv[32g+k] = v[4k+g]
            vk_sb = work.tile([128, NB], F16, tag="vk", bufs=3)
            nc.vector.stream_shuffle(vk_sb, iv_sb, mask=[k] * 32)
            # alternate engines: Pool 2-input ops are ~2 cyc/elem, DVE
            # fp16 is ~0.5 — split so neither starves the W1 consumer
            eng = nc.gpsimd if k % 2 == 0 else nc.vector
            eng.tensor_mul(x3[:, k, :], vk_sb, u4_sb)
        return x3

    # Tail split into 3 stages offset one chunk each, so every matmul
    # that waits on an ACT relu sits in the PE FIFO behind other chunks'
    # matmuls instead of stalling the engine:
    #   tail_a(i): W1 (m-outer) + relus + W2 + relu -> h2
    #   tail_b(i): W3 + relu -> h3          (one chunk later)
    #   tail_c(i): W4 + o_add + store       (two chunks later)
    h2s = [None] * NCHUNK
    h3s = [None] * NCHUNK

    def tail_a(i, x3):
        h1 = work.tile([128, 4, NB], F16, tag="h1")
        for m in range(4):
            h1_ps = psum.tile([128, NB], F32, tag="h1ps", bufs=4)
            for k in range(KC):
                nc.tensor.matmul(h1_ps, w1_sb[:, m, k, :], x3[:, k, :],
                                 start=(k == 0), stop=(k == KC - 1))
            nc.scalar.activation(h1[:, m, :], h1_ps, AF.Relu,
                                 bias=b1_sb[:, m:m + 1])
        h2_ps = psum.tile([128, NB], F32, tag="tail", bufs=2)
        for k in range(4):
            nc.tensor.matmul(h2_ps, w2_sb[:, k, :], h1[:, k, :],
                             start=(k == 0), stop=(k == 3))
        h2 = work.tile([128, NB], F16, tag="h2")
        nc.scalar.activation(h2, h2_ps, AF.Relu, bias=b2_sb[:, 0:1])
        h2s[i] = h2

    def tail_b(i):
        h3_ps = psum.tile([128, NB], F32, tag="tail", bufs=2)
        nc.tensor.matmul(h3_ps, w3_sb, h2s[i], start=True, stop=True)
        h3 = work.tile([128, NB], F16, tag="h3")
        nc.scalar.activation(h3, h3_ps, AF.Relu, bias=b3_sb[:, 0:1])
        h3s[i] = h3

    def tail_c(i):
        bs = slice(i * NB, (i + 1) * NB)
        o_ps = psum.tile([1, NB], F32, tag="tail", bufs=2)
        nc.tensor.matmul(o_ps, w4_sb, h3s[i], start=True, stop=True)
        o_sb = work.tile([1, NB], F32, tag="osb")
        nc.vector.tensor_scalar_add(o_sb, o_ps, b4_val)
        # contiguous [1, NB] row store
        eng = nc.sync if CFG["store_eng"] == "sync" else nc.gpsimd
        eng.dma_start(out_d[:, bs], o_sb)

    x3s = [None] * NCHUNK
    if CFG["tail_split"]:
        for i in range(NCHUNK):
            x3s[i] = head(i)
            if i >= 1:
                tail_a(i - 1, x3s[i - 1])
            if i >= 2:
                tail_b(i - 2)
            if i >= 3:
                tail_c(i - 3)
        tail_a(NCHUNK - 1, x3s[NCHUNK - 1])
        tail_b(NCHUNK - 2)
        tail_c(NCHUNK - 3)
        tail_b(NCHUNK - 1)
        tail_c(NCHUNK - 2)
        tail_c(NCHUNK - 1)
    else:
        for i in range(NCHUNK):
            x3s[i] = head(i)
            if i >= 1:
                tail_a(i - 1, x3s[i - 1])
                tail_b(i - 1)
                tail_c(i - 1)
        tail_a(NCHUNK - 1, x3s[NCHUNK - 1])
        tail_b(NCHUNK - 1)
        tail_c(NCHUNK - 1)


def _prep_consts(var_1, var_2, W1, b1, W2, b2, W3, b3, W4):
    f16 = np.float16
    v1r = np.tile(var_1.T, (1, 4))  # [1024, 128] -> U4 = tile(u, 4)
    v1r_l = np.ascontiguousarray(
        v1r.reshape(KC, 128, 128).transpose(1, 0, 2)).astype(f16)

    # permuted var2 stationary: iv[32g + k] = v[4k + g]
    v2p = np.zeros((D, 128), np.float32)
    for g in range(4):
        for k in range(KC):
            v2p[:, 32 * g + k] = var_2[:, 4 * k + g]
    v2p_l = np.ascontiguousarray(
        v2p.reshape(KC, 128, 128).transpose(1, 0, 2)).astype(f16)

    # w1 m-major: [m, p, k, c] = W1[128k + p, 128m + c]
    w1_l = np.ascontiguousarray(
        W1.reshape(KC, 128, 4, 128).transpose(2, 1, 0, 3)).astype(f16)
    w2_l = np.ascontiguousarray(
        W2.reshape(4, 128, 128).transpose(1, 0, 2)).astype(f16)
    w3_l = np.zeros((128, 128), f16)
    w3_l[:, :R] = W3.astype(f16)
    w4_l = np.zeros((128, 1), f16)
    w4_l[:R, :] = W4.astype(f16)

    b1_l = np.ascontiguousarray(b1.reshape(4, 128).T).astype(np.float32)
    b2_l = b2.reshape(128, 1).astype(np.float32)
    b3_l = np.zeros((128, 1), np.float32)
    b3_l[:R, 0] = b3
    return dict(v1r=v1r_l, v2p=v2p_l, w1l=w1_l, w2l=w2_l,
                w3l=w3_l, w4l=w4_l, b1l=b1_l, b2l=b2_l, b3l=b3_l)


def make_in_maps(**inputs):
    """Shard inputs across cores; returns (in_maps, b4_val)."""
    x1 = np.asarray(inputs["x1"], np.float32)
    x2 = np.asarray(inputs["x2"], np.float32)
    consts = _prep_consts(
        *[np.asarray(inputs[k], np.float32) for k in
          ("var_1", "var_2", "W1", "b1", "W2", "b2", "W3", "b3", "W4")])
    x1h = x1.astype(np.float16)
    x2h = x2.astype(np.float16)

    def shard_t(xh, c):
        # [BL, D] -> [NCHUNK, 128, KC, NB]: [i, p, k, b] = x[i*NB+b, 128k+p]
        xs = xh[c * BL:(c + 1) * BL]  # [BL, D]
        x4 = xs.reshape(NCHUNK, NB, KC, 128)  # [i, b, k, p]
        return np.ascontiguousarray(x4.transpose(0, 3, 2, 1))

    in_maps = []
    for c in range(NCORES):
        m = dict(consts)
        m["x1"] = shard_t(x1h, c)
        m["x2"] = shard_t(x2h, c)
        in_maps.append(m)
    return in_maps, float(np.asarray(inputs["b4"]).reshape(-1)[0])


def run(trace=False, **inputs):
    in_maps, b4_val = make_in_maps(**inputs)
    nc = _build(b4_val)
    res = bass_utils.run_bass_kernel_spmd(
        nc, in_maps, core_ids=list(range(NCORES)), trace=trace)
    out = np.concatenate([r["out"].reshape(BL, 1) for r in res.results],
                         axis=0)
    return out.astype(np.float32), res


def kernel(**inputs):
    out, _ = run(trace=False, **inputs)
    return out


# revision 33
# speedup vs baseline: 1.0333x; 1.0333x over previous
"""Trainium2 Bass kernel for nn_Chemical_feature_interaction.

Math (per sample b):
    u = x1 @ var_1.T                  # [B, 32]
    v = x2 @ var_2                    # [B, 32]
    x3[b, c*32+r] = v[b,c] * u[b,r]   # [B, 1024]
    out = MLP(x3)  (1024->512->128->32->1, ReLU between, bias everywhere)

v3 dataflow (per core, feature-major, fp16 operands, fp32 PSUM):
  - batch sharded 8 ways (4096 rows/core), chunks of NB=512 batch columns.
  - U4 = tile(u,4) [128, NB] from one matmul chain against the tiled
    var1 stationary (M=128 costs the same as M=32).
  - iv [128, NB] from the v-matmul against a host-PERMUTED var2
    stationary: iv[32g+k] = v[4k+g].  Each V_k [128, NB] (= v[4k+p//32])
    is then ONE DVE stream_shuffle with mask [k]*32 — no PE selector
    matmuls (K=32 matmuls are also hw-penalized ~1.7x vs K=128).
  - x3[:,k,:] = V_k * U4 alternating DVE / Pool engines.
  - W1 m-outer (each m-block's ReLU overlaps the next block's matmuls),
    ReLU+bias fused in the PSUM->SBUF copy on ACT.
  - all input DMA pre-emitted in priority order (DMA is ~one serial
    358 GB/s resource): v1r, x1t0, v2p, x2t0, x1t1, w1-m0, x2t1, rest;
    x1 rides the SP ring, x2 the SWDGE queue, weights the ACT ring.
    w1 is m-major in dram so m-blocks land in consumption order.
  - last two chunks' tails interleaved so the ACT<->PE ping-pong of the
    final W2/W3/W4 stage hides under the other chunk's matmuls.
  - out stored per chunk as a contiguous [1, NB] f32 row (out dram is
    (1, BL); host reshapes).

PE work: 54 matmuls/chunk x ~(512+128+~40)cyc @2.4GHz ~= 15.3us/chunk,
8 chunks ~= 123us + edges.  (fp16 selector baseline: 164.6us measured;
v2: 136.5us measured.)
"""

import numpy as np

import concourse.bacc as bacc
import concourse.mybir as mybir
import concourse.tile as tile
from concourse import bass_utils

B = 32768
D = 1024
R = 32
NCORES = 8
BL = B // NCORES  # rows per core
NB = 512  # batch columns per chunk
NCHUNK = BL // NB
KC = D // 128  # k-chunks of the 1024 contraction dims

F16 = mybir.dt.float16
F32 = mybir.dt.float32
AF = mybir.ActivationFunctionType

CFG = {
    "store_eng": "gpsimd",  # sync | gpsimd
    "uv_interleave": False,
}


def _build(b4_val: float, repeat: int = 1):
    """Build the per-core Bass program. repeat>1 wraps the kernel in a
    device-side loop (benchmarking only)."""
    nc = bacc.Bacc("TRN2", target_bir_lowering=False, debug=False)

    # x1/x2 host-pre-transposed and chunk-blocked: [NCHUNK, 128, KC, NB],
    # [i, p, k, b] = x[i*NB + b, 128k + p] — one contiguous 1 MB chunk tile.
    x1_d = nc.dram_tensor("x1", (NCHUNK, 128, KC, NB), F16,
                          kind="ExternalInput").ap()
    x2_d = nc.dram_tensor("x2", (NCHUNK, 128, KC, NB), F16,
                          kind="ExternalInput").ap()
    v1r_d = nc.dram_tensor("v1r", (128, KC, 128), F16, kind="ExternalInput").ap()
    v2p_d = nc.dram_tensor("v2p", (128, KC, 128), F16, kind="ExternalInput").ap()
    w1_d = nc.dram_tensor("w1l", (4, 128, KC, 128), F16,
                          kind="ExternalInput").ap()
    w2_d = nc.dram_tensor("w2l", (128, 4, 128), F16, kind="ExternalInput").ap()
    w3_d = nc.dram_tensor("w3l", (128, 128), F16, kind="ExternalInput").ap()
    w4_d = nc.dram_tensor("w4l", (128, 1), F16, kind="ExternalInput").ap()
    b1_d = nc.dram_tensor("b1l", (128, 4), F32, kind="ExternalInput").ap()
    b2_d = nc.dram_tensor("b2l", (128, 1), F32, kind="ExternalInput").ap()
    b3_d = nc.dram_tensor("b3l", (128, 1), F32, kind="ExternalInput").ap()
    out_d = nc.dram_tensor("out", (1, BL), F32, kind="ExternalOutput").ap()

    with tile.TileContext(nc) as tc:
        with (
            tc.tile_pool(name="consts", bufs=1) as consts,
            tc.tile_pool(name="io", bufs=2) as io,
            tc.tile_pool(name="work", bufs=2) as work,
            tc.tile_pool(name="psum", bufs=2, space="PSUM") as psum,
        ):
            v1r_sb = consts.tile([128, KC, 128], F16)
            v2p_sb = consts.tile([128, KC, 128], F16)
            w1_sb = consts.tile([128, 4, KC, 128], F16)
            w2_sb = consts.tile([128, 4, 128], F16)
            w3_sb = consts.tile([128, 128], F16)
            w4_sb = consts.tile([128, 1], F16)
            b1_sb = consts.tile([128, 4], F32)
            b2_sb = consts.tile([128, 1], F32)
            b3_sb = consts.tile([128, 1], F32)

            consts_sb = dict(v1r=v1r_sb, v2p=v2p_sb, w1=w1_sb, w2=w2_sb,
                             w3=w3_sb, w4=w4_sb, b1=b1_sb, b2=b2_sb,
                             b3=b3_sb)
            consts_d = dict(v1r=v1r_d, v2p=v2p_d, w1=w1_d, w2=w2_d,
                            w3=w3_d, w4=w4_d, b1=b1_d, b2=b2_d, b3=b3_d)

            def body():
                _chunks(nc, x1_d, x2_d, out_d, consts_sb, consts_d, b4_val,
                        io, work, psum)

            if repeat == 1:
                body()
            else:
                ET = mybir.EngineType
                with tc.For_i(0, repeat, 1,
                              hint_engines=(ET.PE, ET.Activation, ET.SP,
                                            ET.DVE, ET.Pool)):
                    body()

    nc.compile()
    return nc


def _chunks(nc, x1_d, x2_d, out_d, csb, cd, b4_val, io, work, psum):
    """Emit input DMA (priority-ordered), then the 8 software-pipelined
    batch chunks (head(i+1) emitted before tail(i))."""

    # ---- input DMA, global priority order (shared serial DMA resource).
    # SP ring: x1 chunks; ACT ring: weights; SWDGE: x2 chunks + stores.
    x1ts, x2ts = [], []
    for i in range(NCHUNK):
        x1ts.append(io.tile([128, KC, NB], F16, tag="x1t", name=f"x1t{i}"))
        x2ts.append(io.tile([128, KC, NB], F16, tag="x2t", name=f"x2t{i}"))

    # Startup priority via the two HWDGE rings (SP carries x1, ACT
    # carries x2 + weights).  The shared issue track round-robins the
    # rings, so pairing entries yields the pipe order:
    #   v1r, v2p, x1_0a, x2_0a, x1_0b, x2_0b, x1t1, x2t1, w1m0..3, ...
    # io bufs=2 makes chunk>=2 loads wait on tile reuse, keeping them
    # from jumping ahead of the w1/bias consts.
    nc.sync.dma_start(csb["v1r"][:], cd["v1r"][:])
    nc.scalar.dma_start(csb["v2p"][:], cd["v2p"][:])
    for sl in ((0, 2), (2, 4), (4, 8)):
        nc.sync.dma_start(x1ts[0][:, sl[0]:sl[1], :],
                          x1_d[0, :, sl[0]:sl[1], :])
        nc.scalar.dma_start(x2ts[0][:, sl[0]:sl[1], :],
                            x2_d[0, :, sl[0]:sl[1], :])
    nc.sync.dma_start(x1ts[1][:], x1_d[1])
    nc.scalar.dma_start(x2ts[1][:], x2_d[1])
    nc.scalar.dma_start(csb["w1"][:, 0], cd["w1"][0])
    nc.scalar.dma_start(csb["b1"][:], cd["b1"][:])
    for m in range(1, 4):
        nc.scalar.dma_start(csb["w1"][:, m], cd["w1"][m])
    nc.scalar.dma_start(x2ts[2][:], x2_d[2])
    nc.sync.dma_start(x1ts[2][:], x1_d[2])
    for name in ("w2", "w3", "w4", "b2", "b3"):
        nc.scalar.dma_start(csb[name][:], cd[name][:])
    for i in range(3, NCHUNK):
        nc.sync.dma_start(x1ts[i][:], x1_d[i])
        nc.scalar.dma_start(x2ts[i][:], x2_d[i])

    # ---- compute
    w1_sb, w2_sb, w3_sb, w4_sb = csb["w1"], csb["w2"], csb["w3"], csb["w4"]
    b1_sb, b2_sb, b3_sb = csb["b1"], csb["b2"], csb["b3"]

    def head(i):
        x1t, x2t = x1ts[i], x2ts[i]
        u4_ps = psum.tile([128, NB], F32, tag="uv", bufs=2, name="u4ps")
        iv_ps = psum.tile([128, NB], F32, tag="uv", bufs=2, name="ivps")
        # k-interleaved accumulation: consumes x slices in DMA arrival
        # order, so chunk-0 compute tracks the incoming stream
        if CFG["uv_interleave"]:
            for k in range(KC):
                nc.tensor.matmul(u4_ps, csb["v1r"][:, k, :], x1t[:, k, :],
                                 start=(k == 0), stop=(k == KC - 1))
                nc.tensor.matmul(iv_ps, csb["v2p"][:, k, :], x2t[:, k, :],
                                 start=(k == 0), stop=(k == KC - 1))
        else:
            for k in range(KC):
                nc.tensor.matmul(u4_ps, csb["v1r"][:, k, :], x1t[:, k, :],
                                 start=(k == 0), stop=(k == KC - 1))
            for k in range(KC):
                nc.tensor.matmul(iv_ps, csb["v2p"][:, k, :], x2t[:, k, :],
                                 start=(k == 0), stop=(k == KC - 1))

        # keep ACT pure-Relu (function switches cost ~1.3us table reloads)
        u4_sb = work.tile([128, NB], F16, tag="u4")
        iv_sb = work.tile([128, NB], F16, tag="iv")
        nc.vector.tensor_copy(iv_sb, iv_ps)
        nc.vector.tensor_copy(u4_sb, u4_ps)

        x3 = work.tile([128, KC, NB], F16, tag="x3")
        for k in range(KC):
            # V_k[32g+j] = iv[32g+k] = v[4k+g]
            vk_sb = work.tile([128, NB], F16, tag="vk", bufs=3)
            nc.vector.stream_shuffle(vk_sb, iv_sb, mask=[k] * 32)
            # alternate engines: Pool 2-input ops are ~2 cyc/elem, DVE
            # fp16 is ~0.5 — split so neither starves the W1 consumer
            eng = nc.gpsimd if k % 2 == 0 else nc.vector
            eng.tensor_mul(x3[:, k, :], vk_sb, u4_sb)
        return x3

    # Tail split into 3 stages offset one chunk each, so every matmul
    # that waits on an ACT relu sits in the PE FIFO behind other chunks'
    # matmuls instead of stalling the engine:
    #   tail_a(i): W1 (m-outer) + relus + W2 + relu -> h2
    #   tail_b(i): W3 + relu -> h3          (one chunk later)
    #   tail_c(i): W4 + o_add + store       (two chunks later)
    h2s = [None] * NCHUNK
    h3s = [None] * NCHUNK

    def tail_a(i, x3, inject=None):
        # W1 m-outer with W2's m-th contraction matmul interleaved right
        # after relu(m): by the last m-block only one W2 matmul remains
        h1 = work.tile([128, 4, NB], F16, tag="h1")
        h2_ps = psum.tile([128, NB], F32, tag="tail", bufs=2)
        for m in range(4):
            h1_ps = psum.tile([128, NB], F32, tag="h1ps", bufs=4)
            for k in range(KC):
                nc.tensor.matmul(h1_ps, w1_sb[:, m, k, :], x3[:, k, :],
                                 start=(k == 0), stop=(k == KC - 1))
            nc.scalar.activation(h1[:, m, :], h1_ps, AF.Relu,
                                 bias=b1_sb[:, m:m + 1])
            nc.tensor.matmul(h2_ps, w2_sb[:, m, :], h1[:, m, :],
                             start=(m == 0), stop=(m == 3))
            if m == 0 and inject is not None:
                inject()
        h2 = work.tile([128, NB], F16, tag="h2")
        nc.scalar.activation(h2, h2_ps, AF.Relu, bias=b2_sb[:, 0:1])
        h2s[i] = h2

    def tail_b(i):
        h3_ps = psum.tile([128, NB], F32, tag="tail", bufs=2)
        nc.tensor.matmul(h3_ps, w3_sb, h2s[i], start=True, stop=True)
        h3 = work.tile([128, NB], F16, tag="h3")
        nc.scalar.activation(h3, h3_ps, AF.Relu, bias=b3_sb[:, 0:1])
        h3s[i] = h3

    def tail_c(i):
        bs = slice(i * NB, (i + 1) * NB)
        o_ps = psum.tile([1, NB], F32, tag="tail", bufs=2)
        nc.tensor.matmul(o_ps, w4_sb, h3s[i], start=True, stop=True)
        o_sb = work.tile([1, NB], F32, tag="osb")
        nc.vector.tensor_scalar_add(o_sb, o_ps, b4_val)
        # contiguous [1, NB] row store
        eng = nc.sync if CFG["store_eng"] == "sync" else nc.gpsimd
        eng.dma_start(out_d[:, bs], o_sb)

    def tail_bc_split(i):
        # final chunk: half-N W3/W4/store chains pipeline across engines,
        # compressing the drain latency
        NH = NB // 2
        for h in range(2):
            cs = slice(h * NH, (h + 1) * NH)
            h3_ps = psum.tile([128, NH], F32, tag="tail", bufs=2)
            nc.tensor.matmul(h3_ps, w3_sb, h2s[i][:, cs],
                             start=True, stop=True)
            h3 = work.tile([128, NH], F16, tag="h3h")
            nc.scalar.activation(h3, h3_ps, AF.Relu, bias=b3_sb[:, 0:1])
            o_ps = psum.tile([1, NH], F32, tag="tail", bufs=2)
            nc.tensor.matmul(o_ps, w4_sb, h3, start=True, stop=True)
            o_sb = work.tile([1, NH], F32, tag="osbh")
            nc.vector.tensor_scalar_add(o_sb, o_ps, b4_val)
            bs = slice(i * NB + h * NH, i * NB + (h + 1) * NH)
            eng = nc.sync if CFG["store_eng"] == "sync" else nc.gpsimd
            eng.dma_start(out_d[:, bs], o_sb)

    x3s = [None] * NCHUNK
    for i in range(NCHUNK):
        x3s[i] = head(i)
        if i >= 2:
            tail_b(i - 2)
        if i >= 1:
            tail_a(i - 1, x3s[i - 1])
        if i >= 2:
            tail_c(i - 2)
    tail_a(NCHUNK - 1, x3s[NCHUNK - 1],
           inject=lambda: (tail_b(NCHUNK - 2), tail_c(NCHUNK - 2)))
    tail_bc_split(NCHUNK - 1)


def _prep_consts(var_1, var_2, W1, b1, W2, b2, W3, b3, W4):
    f16 = np.float16
    v1r = np.tile(var_1.T, (1, 4))  # [1024, 128] -> U4 = tile(u, 4)
    v1r_l = np.ascontiguousarray(
        v1r.reshape(KC, 128, 128).transpose(1, 0, 2)).astype(f16)

    # permuted var2 stationary: iv[32g + k] = v[4k + g]
    v2p = np.zeros((D, 128), np.float32)
    for g in range(4):
        for k in range(KC):
            v2p[:, 32 * g + k] = var_2[:, 4 * k + g]
    v2p_l = np.ascontiguousarray(
        v2p.reshape(KC, 128, 128).transpose(1, 0, 2)).astype(f16)

    # w1 m-major: [m, p, k, c] = W1[128k + p, 128m + c]
    w1_l = np.ascontiguousarray(
        W1.reshape(KC, 128, 4, 128).transpose(2, 1, 0, 3)).astype(f16)
    w2_l = np.ascontiguousarray(
        W2.reshape(4, 128, 128).transpose(1, 0, 2)).astype(f16)
    w3_l = np.zeros((128, 128), f16)
    w3_l[:, :R] = W3.astype(f16)
    w4_l = np.zeros((128, 1), f16)
    w4_l[:R, :] = W4.astype(f16)

    b1_l = np.ascontiguousarray(b1.reshape(4, 128).T).astype(np.float32)
    b2_l = b2.reshape(128, 1).astype(np.float32)
    b3_l = np.zeros((128, 1), np.float32)
    b3_l[:R, 0] = b3
    return dict(v1r=v1r_l, v2p=v2p_l, w1l=w1_l, w2l=w2_l,
                w3l=w3_l, w4l=w4_l, b1l=b1_l, b2l=b2_l, b3l=b3_l)


def make_in_maps(**inputs):
    """Shard inputs across cores; returns (in_maps, b4_val)."""
    x1 = np.asarray(inputs["x1"], np.float32)
    x2 = np.asarray(inputs["x2"], np.float32)
    consts = _prep_consts(
        *[np.asarray(inputs[k], np.float32) for k in
          ("var_1", "var_2", "W1", "b1", "W2", "b2", "W3", "b3", "W4")])
    x1h = x1.astype(np.float16)
    x2h = x2.astype(np.float16)

    def shard_t(xh, c):
        # [BL, D] -> [NCHUNK, 128, KC, NB]: [i, p, k, b] = x[i*NB+b, 128k+p]
        xs = xh[c * BL:(c + 1) * BL]  # [BL, D]
        x4 = xs.reshape(NCHUNK, NB, KC, 128)  # [i, b, k, p]
        return np.ascontiguousarray(x4.transpose(0, 3, 2, 1))

    in_maps = []
    for c in range(NCORES):
        m = dict(consts)
        m["x1"] = shard_t(x1h, c)
        m["x2"] = shard_t(x2h, c)
        in_maps.append(m)
    return in_maps, float(np.asarray(inputs["b4"]).reshape(-1)[0])


def run(trace=False, **inputs):
    in_maps, b4_val = make_in_maps(**inputs)
    nc = _build(b4_val)
    res = bass_utils.run_bass_kernel_spmd(
        nc, in_maps, core_ids=list(range(NCORES)), trace=trace)
    out = np.concatenate([r["out"].reshape(BL, 1) for r in res.results],
                         axis=0)
    return out.astype(np.float32), res


def kernel(**inputs):
    out, _ = run(trace=False, **inputs)
    return out


# revision 34
# speedup vs baseline: 1.0754x; 1.0407x over previous
"""Trainium2 Bass kernel for nn_Chemical_feature_interaction.

Math (per sample b):
    u = x1 @ var_1.T                  # [B, 32]
    v = x2 @ var_2                    # [B, 32]
    x3[b, c*32+r] = v[b,c] * u[b,r]   # [B, 1024]
    out = MLP(x3)  (1024->512->128->32->1, ReLU between, bias everywhere)

v3 dataflow (per core, feature-major, fp16 operands, fp32 PSUM):
  - batch sharded 8 ways (4096 rows/core), chunks of NB=512 batch columns.
  - U4 = tile(u,4) [128, NB] from one matmul chain against the tiled
    var1 stationary (M=128 costs the same as M=32).
  - iv [128, NB] from the v-matmul against a host-PERMUTED var2
    stationary: iv[32g+k] = v[4k+g].  Each V_k [128, NB] (= v[4k+p//32])
    is then ONE DVE stream_shuffle with mask [k]*32 — no PE selector
    matmuls (K=32 matmuls are also hw-penalized ~1.7x vs K=128).
  - x3[:,k,:] = V_k * U4 alternating DVE / Pool engines.
  - W1 m-outer (each m-block's ReLU overlaps the next block's matmuls),
    ReLU+bias fused in the PSUM->SBUF copy on ACT.
  - all input DMA pre-emitted in priority order (DMA is ~one serial
    358 GB/s resource): v1r, x1t0, v2p, x2t0, x1t1, w1-m0, x2t1, rest;
    x1 rides the SP ring, x2 the SWDGE queue, weights the ACT ring.
    w1 is m-major in dram so m-blocks land in consumption order.
  - tails split into 3 stages offset one chunk each (W1+W2 / W3 / W4+
    store) so matmuls that wait on an ACT relu sit in the PE FIFO behind
    other chunks' work; W2's m-th matmul is interleaved into the W1
    m-loop right after relu(m); the last chunk's W3/W4/store chain runs
    on half-N pieces to compress the drain.
  - out stored per chunk as a contiguous [1, NB] f32 row (out dram is
    (1, BL); host reshapes) — a [NB,1] store would be 512 4-byte
    descriptors.

PE work: 54 matmuls/chunk x ~(512 stream + 128 ldweights + ~40)cyc
@2.4GHz ~= 15.3us/chunk, 8 chunks ~= 123us + edges.  Measured (axon
trn2, wall-clock differencing of a device-side repeat loop, noise
+-3us): ~135us per pass vs 164.6us for the selector-matmul baseline.
Notes for future tuning: hw time ~= TimelineSim + 23us ldweights tax
(the sim charges Ldweights 0); K=32 matmuls are hw-penalized (~494ns vs
284ns at K=128, N=512) so the K=32 selector matmuls were replaced by
stream_shuffles; fp8 DoubleRow measures ~(N + 2*128)cyc per instr on hw
(2x FLOPs/instr, NOT the cost model's 4x), which makes accuracy-
preserving hi/lo-split fp8 strictly slower than fp16 here; no ldweights
elision exists for repeated stationaries (measured).
"""

import numpy as np

import concourse.bacc as bacc
import concourse.mybir as mybir
import concourse.tile as tile
from concourse import bass_utils

B = 32768
D = 1024
R = 32
NCORES = 8
BL = B // NCORES  # rows per core
NB = 512  # batch columns per chunk
NCHUNK = BL // NB
KC = D // 128  # k-chunks of the 1024 contraction dims

F16 = mybir.dt.float16
F32 = mybir.dt.float32
AF = mybir.ActivationFunctionType

CFG = {
    "store_eng": "gpsimd",  # sync | gpsimd
    "uv_interleave": False,
}


def _build(b4_val: float, repeat: int = 1):
    """Build the per-core Bass program. repeat>1 wraps the kernel in a
    device-side loop (benchmarking only)."""
    nc = bacc.Bacc("TRN2", target_bir_lowering=False, debug=False)

    # x1/x2 host-pre-transposed and chunk-blocked: [NCHUNK, 128, KC, NB],
    # [i, p, k, b] = x[i*NB + b, 128k + p] — one contiguous 1 MB chunk tile.
    x1_d = nc.dram_tensor("x1", (NCHUNK, 128, KC, NB), F16,
                          kind="ExternalInput").ap()
    x2_d = nc.dram_tensor("x2", (NCHUNK, 128, KC, NB), F16,
                          kind="ExternalInput").ap()
    v1r_d = nc.dram_tensor("v1r", (128, KC, 128), F16, kind="ExternalInput").ap()
    v2p_d = nc.dram_tensor("v2p", (128, KC, 128), F16, kind="ExternalInput").ap()
    w1_d = nc.dram_tensor("w1l", (4, 128, KC, 128), F16,
                          kind="ExternalInput").ap()
    w2_d = nc.dram_tensor("w2l", (128, 4, 128), F16, kind="ExternalInput").ap()
    w3_d = nc.dram_tensor("w3l", (128, 128), F16, kind="ExternalInput").ap()
    w4_d = nc.dram_tensor("w4l", (128, 1), F16, kind="ExternalInput").ap()
    b1_d = nc.dram_tensor("b1l", (128, 4), F32, kind="ExternalInput").ap()
    b2_d = nc.dram_tensor("b2l", (128, 1), F32, kind="ExternalInput").ap()
    b3_d = nc.dram_tensor("b3l", (128, 1), F32, kind="ExternalInput").ap()
    out_d = nc.dram_tensor("out", (1, BL), F32, kind="ExternalOutput").ap()

    with tile.TileContext(nc) as tc:
        with (
            tc.tile_pool(name="consts", bufs=1) as consts,
            tc.tile_pool(name="io", bufs=2) as io,
            tc.tile_pool(name="work", bufs=2) as work,
            tc.tile_pool(name="psum", bufs=2, space="PSUM") as psum,
        ):
            v1r_sb = consts.tile([128, KC, 128], F16)
            v2p_sb = consts.tile([128, KC, 128], F16)
            w1_sb = consts.tile([128, 4, KC, 128], F16)
            w2_sb = consts.tile([128, 4, 128], F16)
            w3_sb = consts.tile([128, 128], F16)
            w4_sb = consts.tile([128, 1], F16)
            b1_sb = consts.tile([128, 4], F32)
            b2_sb = consts.tile([128, 1], F32)
            b3_sb = consts.tile([128, 1], F32)

            consts_sb = dict(v1r=v1r_sb, v2p=v2p_sb, w1=w1_sb, w2=w2_sb,
                             w3=w3_sb, w4=w4_sb, b1=b1_sb, b2=b2_sb,
                             b3=b3_sb)
            consts_d = dict(v1r=v1r_d, v2p=v2p_d, w1=w1_d, w2=w2_d,
                            w3=w3_d, w4=w4_d, b1=b1_d, b2=b2_d, b3=b3_d)

            def body():
                _chunks(nc, x1_d, x2_d, out_d, consts_sb, consts_d, b4_val,
                        io, work, psum)

            if repeat == 1:
                body()
            else:
                ET = mybir.EngineType
                with tc.For_i(0, repeat, 1,
                              hint_engines=(ET.PE, ET.Activation, ET.SP,
                                            ET.DVE, ET.Pool)):
                    body()

    nc.compile()
    return nc


def _chunks(nc, x1_d, x2_d, out_d, csb, cd, b4_val, io, work, psum):
    """Emit input DMA (priority-ordered), then the 8 software-pipelined
    batch chunks (head(i+1) emitted before tail(i))."""

    # ---- input DMA, global priority order (shared serial DMA resource).
    # SP ring: x1 chunks; ACT ring: weights; SWDGE: x2 chunks + stores.
    x1ts, x2ts = [], []
    for i in range(NCHUNK):
        x1ts.append(io.tile([128, KC, NB], F16, tag="x1t", name=f"x1t{i}"))
        x2ts.append(io.tile([128, KC, NB], F16, tag="x2t", name=f"x2t{i}"))

    # Startup priority via the two HWDGE rings (SP carries x1, ACT
    # carries x2 + weights).  The shared issue track round-robins the
    # rings, so pairing entries yields the pipe order:
    #   v1r, v2p, x1_0a, x2_0a, x1_0b, x2_0b, x1t1, x2t1, w1m0..3, ...
    # io bufs=2 makes chunk>=2 loads wait on tile reuse, keeping them
    # from jumping ahead of the w1/bias consts.
    nc.sync.dma_start(csb["v1r"][:], cd["v1r"][:])
    nc.scalar.dma_start(csb["v2p"][:], cd["v2p"][:])
    for sl in ((0, 2), (2, 4), (4, 8)):
        nc.sync.dma_start(x1ts[0][:, sl[0]:sl[1], :],
                          x1_d[0, :, sl[0]:sl[1], :])
        nc.scalar.dma_start(x2ts[0][:, sl[0]:sl[1], :],
                            x2_d[0, :, sl[0]:sl[1], :])
    nc.sync.dma_start(x1ts[1][:], x1_d[1])
    nc.scalar.dma_start(x2ts[1][:], x2_d[1])
    nc.scalar.dma_start(csb["w1"][:, 0], cd["w1"][0])
    nc.scalar.dma_start(csb["b1"][:], cd["b1"][:])
    for m in range(1, 4):
        nc.scalar.dma_start(csb["w1"][:, m], cd["w1"][m])
    nc.scalar.dma_start(x2ts[2][:], x2_d[2])
    nc.sync.dma_start(x1ts[2][:], x1_d[2])
    for name in ("w2", "w3", "w4", "b2", "b3"):
        nc.scalar.dma_start(csb[name][:], cd[name][:])
    for i in range(3, NCHUNK):
        nc.sync.dma_start(x1ts[i][:], x1_d[i])
        nc.scalar.dma_start(x2ts[i][:], x2_d[i])

    # ---- compute
    w1_sb, w2_sb, w3_sb, w4_sb = csb["w1"], csb["w2"], csb["w3"], csb["w4"]
    b1_sb, b2_sb, b3_sb = csb["b1"], csb["b2"], csb["b3"]

    def head(i):
        x1t, x2t = x1ts[i], x2ts[i]
        u4_ps = psum.tile([128, NB], F32, tag="uv", bufs=2, name="u4ps")
        iv_ps = psum.tile([128, NB], F32, tag="uv", bufs=2, name="ivps")
        # k-interleaved accumulation: consumes x slices in DMA arrival
        # order, so chunk-0 compute tracks the incoming stream
        if CFG["uv_interleave"]:
            for k in range(KC):
                nc.tensor.matmul(u4_ps, csb["v1r"][:, k, :], x1t[:, k, :],
                                 start=(k == 0), stop=(k == KC - 1))
                nc.tensor.matmul(iv_ps, csb["v2p"][:, k, :], x2t[:, k, :],
                                 start=(k == 0), stop=(k == KC - 1))
        else:
            for k in range(KC):
                nc.tensor.matmul(u4_ps, csb["v1r"][:, k, :], x1t[:, k, :],
                                 start=(k == 0), stop=(k == KC - 1))
            for k in range(KC):
                nc.tensor.matmul(iv_ps, csb["v2p"][:, k, :], x2t[:, k, :],
                                 start=(k == 0), stop=(k == KC - 1))

        # keep ACT pure-Relu (function switches cost ~1.3us table reloads)
        u4_sb = work.tile([128, NB], F16, tag="u4")
        iv_sb = work.tile([128, NB], F16, tag="iv")
        nc.vector.tensor_copy(iv_sb, iv_ps)
        nc.vector.tensor_copy(u4_sb, u4_ps)

        x3 = work.tile([128, KC, NB], F16, tag="x3")
        for k in range(KC):
            # V_k[32g+j] = iv[32g+k] = v[4k+g]
            vk_sb = work.tile([128, NB], F16, tag="vk", bufs=3)
            nc.vector.stream_shuffle(vk_sb, iv_sb, mask=[k] * 32)
            # alternate engines: Pool 2-input ops are ~2 cyc/elem, DVE
            # fp16 is ~0.5 — split so neither starves the W1 consumer
            eng = nc.gpsimd if k % 2 == 0 else nc.vector
            eng.tensor_mul(x3[:, k, :], vk_sb, u4_sb)
        return x3

    # Tail split into 3 stages offset one chunk each, so every matmul
    # that waits on an ACT relu sits in the PE FIFO behind other chunks'
    # matmuls instead of stalling the engine:
    #   tail_a(i): W1 (m-outer) + relus + W2 + relu -> h2
    #   tail_b(i): W3 + relu -> h3          (one chunk later)
    #   tail_c(i): W4 + o_add + store       (two chunks later)
    h2s = [None] * NCHUNK
    h3s = [None] * NCHUNK

    def tail_a(i, x3, inject=None):
        # W1 m-outer with W2's m-th contraction matmul interleaved right
        # after relu(m): by the last m-block only one W2 matmul remains
        h1 = work.tile([128, 4, NB], F16, tag="h1")
        h2_ps = psum.tile([128, NB], F32, tag="tail", bufs=2)
        for m in range(4):
            h1_ps = psum.tile([128, NB], F32, tag="h1ps", bufs=4)
            for k in range(KC):
                nc.tensor.matmul(h1_ps, w1_sb[:, m, k, :], x3[:, k, :],
                                 start=(k == 0), stop=(k == KC - 1))
            nc.scalar.activation(h1[:, m, :], h1_ps, AF.Relu,
                                 bias=b1_sb[:, m:m + 1])
            nc.tensor.matmul(h2_ps, w2_sb[:, m, :], h1[:, m, :],
                             start=(m == 0), stop=(m == 3))
            if m == 0 and inject is not None:
                inject()
        h2 = work.tile([128, NB], F16, tag="h2")
        nc.scalar.activation(h2, h2_ps, AF.Relu, bias=b2_sb[:, 0:1])
        h2s[i] = h2

    def tail_b(i):
        h3_ps = psum.tile([128, NB], F32, tag="tail", bufs=2)
        nc.tensor.matmul(h3_ps, w3_sb, h2s[i], start=True, stop=True)
        h3 = work.tile([128, NB], F16, tag="h3")
        nc.scalar.activation(h3, h3_ps, AF.Relu, bias=b3_sb[:, 0:1])
        h3s[i] = h3

    def tail_c(i):
        bs = slice(i * NB, (i + 1) * NB)
        o_ps = psum.tile([1, NB], F32, tag="tail", bufs=2)
        nc.tensor.matmul(o_ps, w4_sb, h3s[i], start=True, stop=True)
        o_sb = work.tile([1, NB], F32, tag="osb")
        nc.vector.tensor_scalar_add(o_sb, o_ps, b4_val)
        # contiguous [1, NB] row store
        eng = nc.sync if CFG["store_eng"] == "sync" else nc.gpsimd
        eng.dma_start(out_d[:, bs], o_sb)

    def tail_bc_split(i):
        # final chunk: half-N W3/W4/store chains pipeline across engines,
        # compressing the drain latency
        NH = NB // 2
        for h in range(2):
            cs = slice(h * NH, (h + 1) * NH)
            h3_ps = psum.tile([128, NH], F32, tag="tail", bufs=2)
            nc.tensor.matmul(h3_ps, w3_sb, h2s[i][:, cs],
                             start=True, stop=True)
            h3 = work.tile([128, NH], F16, tag="h3h")
            nc.scalar.activation(h3, h3_ps, AF.Relu, bias=b3_sb[:, 0:1])
            o_ps = psum.tile([1, NH], F32, tag="tail", bufs=2)
            nc.tensor.matmul(o_ps, w4_sb, h3, start=True, stop=True)
            o_sb = work.tile([1, NH], F32, tag="osbh")
            nc.vector.tensor_scalar_add(o_sb, o_ps, b4_val)
            bs = slice(i * NB + h * NH, i * NB + (h + 1) * NH)
            eng = nc.sync if CFG["store_eng"] == "sync" else nc.gpsimd
            eng.dma_start(out_d[:, bs], o_sb)

    x3s = [None] * NCHUNK
    for i in range(NCHUNK):
        x3s[i] = head(i)
        if i >= 2:
            tail_b(i - 2)
        if i >= 1:
            tail_a(i - 1, x3s[i - 1])
        if i >= 2:
            tail_c(i - 2)
    tail_a(NCHUNK - 1, x3s[NCHUNK - 1],
           inject=lambda: (tail_b(NCHUNK - 2), tail_c(NCHUNK - 2)))
    tail_bc_split(NCHUNK - 1)


def _prep_consts(var_1, var_2, W1, b1, W2, b2, W3, b3, W4):
    f16 = np.float16
    v1r = np.tile(var_1.T, (1, 4))  # [1024, 128] -> U4 = tile(u, 4)
    v1r_l = np.ascontiguousarray(
        v1r.reshape(KC, 128, 128).transpose(1, 0, 2)).astype(f16)

    # permuted var2 stationary: iv[32g + k] = v[4k + g]
    v2p = np.zeros((D, 128), np.float32)
    for g in range(4):
        for k in range(KC):
            v2p[:, 32 * g + k] = var_2[:, 4 * k + g]
    v2p_l = np.ascontiguousarray(
        v2p.reshape(KC, 128, 128).transpose(1, 0, 2)).astype(f16)

    # w1 m-major: [m, p, k, c] = W1[128k + p, 128m + c]
    w1_l = np.ascontiguousarray(
        W1.reshape(KC, 128, 4, 128).transpose(2, 1, 0, 3)).astype(f16)
    w2_l = np.ascontiguousarray(
        W2.reshape(4, 128, 128).transpose(1, 0, 2)).astype(f16)
    w3_l = np.zeros((128, 128), f16)
    w3_l[:, :R] = W3.astype(f16)
    w4_l = np.zeros((128, 1), f16)
    w4_l[:R, :] = W4.astype(f16)

    b1_l = np.ascontiguousarray(b1.reshape(4, 128).T).astype(np.float32)
    b2_l = b2.reshape(128, 1).astype(np.float32)
    b3_l = np.zeros((128, 1), np.float32)
    b3_l[:R, 0] = b3
    return dict(v1r=v1r_l, v2p=v2p_l, w1l=w1_l, w2l=w2_l,
                w3l=w3_l, w4l=w4_l, b1l=b1_l, b2l=b2_l, b3l=b3_l)


def make_in_maps(**inputs):
    """Shard inputs across cores; returns (in_maps, b4_val)."""
    x1 = np.asarray(inputs["x1"], np.float32)
    x2 = np.asarray(inputs["x2"], np.float32)
    consts = _prep_consts(
        *[np.asarray(inputs[k], np.float32) for k in
          ("var_1", "var_2", "W1", "b1", "W2", "b2", "W3", "b3", "W4")])
    x1h = x1.astype(np.float16)
    x2h = x2.astype(np.float16)

    def shard_t(xh, c):
        # [BL, D] -> [NCHUNK, 128, KC, NB]: [i, p, k, b] = x[i*NB+b, 128k+p]
        xs = xh[c * BL:(c + 1) * BL]  # [BL, D]
        x4 = xs.reshape(NCHUNK, NB, KC, 128)  # [i, b, k, p]
        return np.ascontiguousarray(x4.transpose(0, 3, 2, 1))

    in_maps = []
    for c in range(NCORES):
        m = dict(consts)
        m["x1"] = shard_t(x1h, c)
        m["x2"] = shard_t(x2h, c)
        in_maps.append(m)
    return in_maps, float(np.asarray(inputs["b4"]).reshape(-1)[0])


def run(trace=False, **inputs):
    in_maps, b4_val = make_in_maps(**inputs)
    nc = _build(b4_val)
    res = bass_utils.run_bass_kernel_spmd(
        nc, in_maps, core_ids=list(range(NCORES)), trace=trace)
    out = np.concatenate([r["out"].reshape(BL, 1) for r in res.results],
                         axis=0)
    return out.astype(np.float32), res


def kernel(**inputs):
    out, _ = run(trace=False, **inputs)
    return out
